# revision 1
# baseline (speedup 1.0000x reference)
"""Trainium2 Bass kernel for the quirky multi-head attention problem.

Math (per batch b, head a), faithful to the reference:
    K = x[b] @ W_K[a].T          # [S, H]
    Q = x[b] @ W_Q[a].T
    V = x[b] @ W_V[a].T
    s[c, C] = (K @ Q.T)[c, C] / sqrt(H)        rows c = "key" index
    valid iff C <= c (tril); softmax over C per row c
    E = exp(s) * tril            # no max-subtraction: |s| <= ~7, fp32-safe
    denom[c] = sum_C E[c, C]
    z[C, h] = sum_c E[c, C] * (V/denom)[c, h]  # = E.T @ (V/denom)
    out[b] += z @ W_O[a].T
Sharding: 8 cores = 2 batches x 4 head-pairs. Each core handles one batch
and two heads; the attention matrix is device-local. Host sums the four
head-pair partial outputs per batch.

Device layouts (per core):
    xp   [128, EC, S] x[b] pre-packed on host: xp[p, ec, s] = x[s, ec*128+p]
    wk/wq/wv [128, EC*128] W[a0].T | W[a1].T, ec-chunks packed on partitions
    wo   [128, E]     W_O[a0].T on partitions 0:64, W_O[a1].T on 64:128
    maskb [128, 128]  additive causal mask band (0 or -1e9)
    outT [E, S]       partial output, transposed

On-chip flow per head: scores [c_blk=128 rows, 512-wide C chunks] are
matmul'd into rotating PSUM wave tiles (2 banks x 2 bufs), the diagonal
gets an additive -1e9 triangle mask, ScalarE applies exp (scale=1/sqrt(H))
writing the row panel to SBUF (fp16) with a fused per-row accumulation
(softmax denominator). z^T accumulates in PSUM across row blocks; chunk j
of C lives at partition half (j < NCH/2 ? 0 : 64) so z^T fits in 4 banks
and coexists with the wave tiles. Matmul dtypes: fp16 operands for the
attention matmuls (4e-4 rel err, 1 col/cycle like bf16), f32r (single-pass
fp32, now fp16 wire) for projections. z matmuls for block cb are emitted Z_LAG blocks
late so PE streams without stalling on the softmax chain; head 1 sweeps
rows in reverse so each z chunk j finalizes at block 4j and its copy +
merged-head output projection trickle through the sweep instead of
bunching in a cold epilogue.

DMA: weights are host-prepacked so each input is ONE large descriptor-
friendly transfer (the DGE generator serializes at ~650ns/copy); x streams
in a narrow 128-col first slice + 384-col second + seven 512-col eighths
so block 0's kt/qt chain starts ~6us in instead of ~16us.

Timing (TimelineSim cost model; HW ~= 1.195x sim per baseline calibration):
baseline 280237ns (HW 334882) -> this kernel 236653ns. x/weights/output
ship as fp16 on the wire (adds ~2e-4 rel err vs the 2e-3 gate, halves the
serialized DMA stream to ~25us total, and removes f32r's 4x penalty on the
narrow N=128 V-projection matmuls: PE busy drops 169->147us). ACT (exp)
busy ~170us is the single bottleneck at ~71% occupancy; remaining slack is
softmax-chain latency in head 0 (wave-buffer double-hold during projection
slices) and the reverse-sweep tail.
"""

import math

import numpy as np

B, S_FULL, E, A, H = 2, 4096, 512, 8, 64
N_CORES = 8
NEG_BIG = -1.0e9

import os as _os

ATTN_DT = _os.environ.get("ATTN_DT", "fp16")
PROJ_DT = _os.environ.get("PROJ_DT", "fp16")
OUT_DT = _os.environ.get("OUT_DT", "fp16")
Z_LAG = int(_os.environ.get("Z_LAG", "2"))
PANEL_BUFS = int(_os.environ.get("PANEL_BUFS", "15"))
ST_POOL = int(_os.environ.get("ST_POOL", "0"))   # out-proj st copies on Pool
KQ_POOL = int(_os.environ.get("KQ_POOL", "0"))   # kt/qt copies on Pool
KQ_ACT = int(_os.environ.get("KQ_ACT", "0"))     # kt/qt copies on ACT
FILL = int(_os.environ.get("FILL", "0"))
SMALL_BUFS = int(_os.environ.get("SMALL_BUFS", "8"))
OUTST_BUFS = int(_os.environ.get("OUTST_BUFS", "4"))         # dummy ldweights per block
OP_POPS = int(_os.environ.get("OP_POPS", "2"))   # out-proj pieces per block

_prog_cache = {}


def _build_program(S, attn_dt=None, proj_dt=None):
    import concourse.mybir as mybir
    import concourse.tile as tile
    from concourse import bacc

    attn_dt = attn_dt or ATTN_DT
    proj_dt = proj_dt or PROJ_DT
    f32 = mybir.dt.float32
    f32r = mybir.dt.float32r
    bf16 = mybir.dt.bfloat16
    fp16 = mybir.dt.float16
    att_store = {"bf16": bf16, "fp16": fp16, "f32r": f32r, "f32": f32}[attn_dt]
    z_store = {"bf16": bf16, "fp16": fp16, "f32r": fp16, "f32": f32}[attn_dt]
    proj_store = {"f32r": f32r, "f32": f32, "fp16": fp16}[proj_dt]
    out_store = {"f32": f32, "fp16": fp16}[OUT_DT]

    EC = E // 128            # e chunks (contraction for projections)
    NCB = S // 128           # row blocks
    NCH = S // 512           # C chunks per full row
    HALF = NCH // 2          # chunks per partition half of z^T
    assert NCH % 2 == 0

    nc = bacc.Bacc("TRN2", target_bir_lowering=False, debug=False)

    xp = nc.dram_tensor("xp", [128, EC, S], proj_store, kind="ExternalInput")
    wk = nc.dram_tensor("wk", [128, EC * 128], proj_store, kind="ExternalInput")
    wq = nc.dram_tensor("wq", [128, EC * 128], proj_store, kind="ExternalInput")
    wv = nc.dram_tensor("wv", [128, EC * 128], proj_store, kind="ExternalInput")
    wo = nc.dram_tensor("wo", [128, E], proj_store, kind="ExternalInput")
    maskb = nc.dram_tensor("maskb", [128, 128], f32, kind="ExternalInput")
    maskm = nc.dram_tensor("maskm", [128, 128], fp16, kind="ExternalInput")
    outT = nc.dram_tensor("outT", [E, S], out_store, kind="ExternalOutput")

    ExpF = mybir.ActivationFunctionType.Exp
    AxX = mybir.AxisListType.X
    AluAdd = mybir.AluOpType.add

    with tile.TileContext(nc) as tc:
        with (
            tc.tile_pool(name="singles", bufs=1) as singles,
            tc.tile_pool(name="panelp", bufs=PANEL_BUFS) as panelp,
            tc.tile_pool(name="small", bufs=SMALL_BUFS) as small,
            tc.tile_pool(name="outst", bufs=OUTST_BUFS) as outst,
            tc.tile_pool(name="ps", bufs=2, space="PSUM") as ps,
            tc.tile_pool(name="zps", bufs=1, space="PSUM") as zps,
        ):
            # ---- load inputs: one big transfer per tensor, x in eighths,
            # ordered so the first projection inputs land first ----
            xt = singles.tile([128, EC, S], proj_store)
            wks = singles.tile([128, EC, 128], proj_store)
            wqs = singles.tile([128, EC, 128], proj_store)
            wvs = singles.tile([128, EC, 128], proj_store)
            wos = singles.tile([128, E], proj_store)
            msk = singles.tile([128, 128], f32)
            mskm = singles.tile([128, 128], fp16)

            nc.sync.dma_start(out=wks[:, :, :], in_=wk[:, :])
            nc.sync.dma_start(out=wqs[:, :, :], in_=wq[:, :])
            nc.sync.dma_start(out=wvs[:, :, :], in_=wv[:, :])
            SQ = S // 8
            # first slice narrow (128 cols) so block 0's kt/qt chain starts
            # ~3us earlier; masks right after it (needed by block 0's exp)
            xsl = [(0, 128), (128, 512)] + [(q * SQ, (q + 1) * SQ) for q in range(1, 8)]
            for i, (lo, hi) in enumerate(xsl):
                nc.sync.dma_start(out=xt[:, :, lo:hi], in_=xp[:, :, lo:hi])
                if i == 0:
                    nc.sync.dma_start(out=msk, in_=maskb[:, :])
                    nc.sync.dma_start(out=mskm, in_=maskm[:, :])
            nc.sync.dma_start(out=wos, in_=wo[:, :])
            # bf16 always: the K=1 zeroing matmuls are invalid ISA in f32r,
            # and mixing dtypes across an accumulation group is fine.
            zero_t = singles.tile([1, 576], bf16)
            nc.vector.memset(zero_t, 0.0)

            # ---- projections (emitted per x-quarter, interleaved into the
            # head-0 narrow blocks so PE stays dense while DMA streams in) ----
            kt = singles.tile([128, S], att_store)
            qt = singles.tile([128, S], att_store)
            vsb = singles.tile([128, NCB, 128], f32)
            CPQ = NCH // 4           # 512-chunks per x quarter

            scr = {"zT": None, "cb": 0, "rot": 0}

            def proj_wt(width):
                # head0 blocks < 4k-1 may scratch in zT bank k: that bank's
                # chunks start accumulating only at block 4k (lower, real
                # start=True) / its upper-half zero group is emitted at block
                # 4k too - both after any scratch use (WAR-ordered by tile)
                if scr["zT"] is not None:
                    avail = [k for k in (1, 2, 3) if scr["cb"] < 4 * k - 1]
                    if avail:
                        k = avail[scr["rot"] % len(avail)]
                        scr["rot"] += 1
                        return scr["zT"][:, k * 512:k * 512 + width], True
                return ps.tile([128, 1024], f32, tag="wave", name="wt")[:, :width], False

            def emit_kq_part(dst, w, lo, hi):
                wt, is_scr = proj_wt(hi - lo)
                for ec in range(EC):
                    nc.tensor.matmul(
                        wt, w[:, ec, :], xt[:, ec, lo:hi],
                        start=(ec == 0), stop=(ec == EC - 1),
                        skip_group_check=is_scr,
                    )
                if KQ_ACT:
                    nc.scalar.copy(dst[:, lo:hi], wt)
                else:
                    cp = nc.gpsimd if KQ_POOL else nc.vector
                    cp.tensor_copy(dst[:, lo:hi], wt)

            def emit_kq_chunk(dst, w, cc):
                emit_kq_part(dst, w, cc * 512, (cc + 1) * 512)

            def emit_v_block(cb):
                bsl = slice(cb * 128, (cb + 1) * 128)
                wt = ps.tile([128, 1024], f32, tag="wave", name="wt")[:, :128]
                for ec in range(EC):
                    nc.tensor.matmul(
                        wt, xt[:, ec, bsl], wvs[:, ec, :],
                        start=(ec == 0), stop=(ec == EC - 1),
                    )
                nc.vector.tensor_copy(vsb[:, cb, :], wt)

            def emit_proj_slice(cb):
                # spread projections across row blocks so the PE surplus
                # never starves ACT of fresh score waves. Block cb needs
                # kt cols <= cb*128+128 and qt chunks <= cb//4, so:
                #   block 0/1: split first chunk (narrow chain -> fast start)
                #   block 8q+2/3: kt/qt chunk 2q+1
                #   block 8q+6/7: kt/qt chunk 2q+2 (ready before block 8q+8)
                #   every block: its own vsb
                q, r = divmod(cb, 8)
                if cb == 0:
                    emit_kq_part(kt, wks, 0, 128)
                    emit_kq_part(qt, wqs, 0, 128)
                    emit_v_block(cb)
                    return
                # V first: its short copy drains while the kq chunk's longer
                # PSUM->SBUF copy overlaps the next block's scores
                emit_v_block(cb)
                if cb == 1:
                    emit_kq_part(kt, wks, 128, 512)
                    emit_kq_part(qt, wqs, 128, 512)
                elif r == 1 and q > 0:
                    emit_kq_chunk(kt, wks, 2 * q + 1)
                elif r == 2 and q == 0:
                    emit_kq_chunk(kt, wks, 1)
                elif r == 2 and q > 0:
                    emit_kq_chunk(qt, wqs, 2 * q + 1)
                elif r == 3 and q == 0:
                    emit_kq_chunk(qt, wqs, 1)
                elif r == 4 and q < 3:
                    emit_kq_chunk(kt, wks, 2 * q + 2)
                elif r == 5 and q < 3:
                    emit_kq_chunk(qt, wqs, 2 * q + 2)

            # zsb holds both heads' z in SBUF: head h on partitions
            # [64h, 64h+64), C chunk j at cols [512j, 512j+512). The output
            # projection then contracts both heads in ONE K=128 matmul.
            zsb = singles.tile([128, S], proj_store)

            # ---- attention per head ----
            for h in range(2):
                hs = slice(h * 64, (h + 1) * 64)
                # head 0 forward, head 1 reverse: in reverse order z chunk j
                # finalizes at block 4j, so its copy + output projection
                # trickle through the sweep
                order = list(range(NCB - 1, -1, -1)) if h == 1 else list(range(NCB))
                zT = zps.tile([128, HALF * 512], f32, name="zT")

                def emit_dummy(k):
                    # zero the upper partition half of z bank k (chunks
                    # j >= HALF accumulate with start=False onto these; a
                    # real start=True there would clear the lower half's
                    # bank bits)
                    nc.tensor.matmul(
                        zT[64:128, k * 512:(k + 1) * 512],
                        zero_t[:, :64], zero_t[:, 64:576],
                        start=True, stop=False, skip_group_check=True,
                    )

                if h == 1:
                    for k in range(HALF):
                        emit_dummy(k)
                else:
                    # head 0: bank k is scratch for projections until block
                    # 4k-2; its dummy is emitted at block 4k instead
                    emit_dummy(0)
                    scr["zT"] = zT

                first_cb = {}
                last_cb = {}
                for j in range(NCH):
                    part = [cb for cb in order if cb >= 4 * j]
                    first_cb[j] = part[0]
                    last_cb[j] = part[-1]

                def copy_zchunk(j):
                    poff = 0 if j < HALF else 64
                    col = (j % HALF) * 512
                    nc.vector.tensor_copy(
                        zsb[h * 64:h * 64 + 64, j * 512:(j + 1) * 512],
                        zT[poff:poff + 64, col:col + 512],
                    )

                def emit_out_piece(ccn, ecn, late=False):
                    col = ccn * 512
                    esl = slice(ecn * 128, (ecn + 1) * 128)
                    if ccn < HALF:
                        # bank ccn of zT is fully copied out by now (chunk
                        # ccn just finished; chunk ccn+HALF finished 16
                        # blocks earlier in the reverse sweep) — reuse it so
                        # out-proj never steals a score wave buffer
                        wt = zT[:, (ccn % HALF) * 512:(ccn % HALF) * 512 + 512]
                    else:
                        wt = ps.tile([128, 1024], f32, tag="wave", name="wt")[:, :512]
                    nc.tensor.matmul(
                        wt, wos[:, esl], zsb[:, col:col + 512],
                        start=True, stop=True, skip_group_check=True,
                    )
                    st = outst.tile([128, 512], out_store, name="st")
                    if late:
                        # tail: ACT is idle once the exp stream dries up, and
                        # DVE's in-order queue would delay the next block's
                        # den chain behind this 658ns copy
                        nc.scalar.copy(st, wt)
                    else:
                        cp = nc.gpsimd if ST_POOL else nc.vector
                        cp.tensor_copy(st, wt)
                    nc.sync.dma_start(out=outT[esl, col:col + 512], in_=st)

                def emit_z(item):
                    vt_i, panel_i, nch_i, cb_i = item
                    for j in range(nch_i):
                        poff = 0 if j < HALF else 64
                        col = (j % HALF) * 512
                        start = (j < HALF) and cb_i == first_cb[j]
                        stop = cb_i == last_cb[j]
                        nc.tensor.matmul(
                            zT[poff:poff + 64, col:col + 512],
                            vt_i,
                            panel_i[:, j * 512:(j + 1) * 512],
                            start=start, stop=stop,
                            skip_group_check=True,
                        )
                    # Head 1 runs in reverse: chunk j is final once cb=4j is
                    # done; trickle its copy + the merged output projection
                    # into the sweep, ONE ecn piece per block so out-proj
                    # never monopolizes the wave buffers.
                    if h == 1:
                        if cb_i % 4 == 0:
                            j = cb_i // 4
                            copy_zchunk(j)
                            out_pieces.extend((j, ecn) for ecn in range(EC))
                        if out_pieces:
                            emit_out_piece(*out_pieces.pop(0), late=cb_i <= 8)

                pending = []
                out_pieces = []
                for oi, cb in enumerate(order):
                    if h == 0:
                        scr["cb"] = cb
                        if cb in (4, 8, 12):
                            emit_dummy(cb // 4)
                        emit_proj_slice(cb)
                    c0 = cb * 128
                    nch = (c0 + 128 + 511) // 512
                    nwaves = (nch + 1) // 2
                    lastw = c0 + 128 - (nch - 1) * 512   # width of diag chunk
                    panel = panelp.tile([128, S], z_store, name="panel")
                    if lastw < 512:
                        # zero the diag chunk tail so z matmuls read zeros
                        nc.gpsimd.memset(
                            panel[:, (nch - 1) * 512 + lastw:nch * 512], 0.0
                        )
                    rsp = small.tile([128, 4], f32, name="rsp")
                    for wv_i in range(nwaves):
                        jlo = 2 * wv_i
                        jhi = min(jlo + 2, nch)
                        wt = ps.tile([128, 1024], f32, tag="wave", name="wt")
                        for j in range(jlo, jhi):
                            w_n = lastw if j == nch - 1 else 512
                            nc.tensor.matmul(
                                wt[:, (j - jlo) * 512:(j - jlo) * 512 + w_n],
                                kt[hs, c0:c0 + 128],
                                qt[hs, j * 512:j * 512 + w_n],
                                start=True, stop=True,
                            )
                        if jhi == nch and nwaves > 1:
                            # mask only the last 128 cols (the true triangle);
                            # earlier diag-chunk cols are fully valid
                            o = c0 - (nch - 1) * 512
                            dlo = (nch - 1 - jlo) * 512 + o
                            nc.vector.tensor_add(
                                wt[:, dlo:dlo + 128], wt[:, dlo:dlo + 128],
                                msk,
                            )
                        wlen = (jhi - jlo - 1) * 512 + (lastw if jhi == nch else 512)
                        nc.scalar.activation(
                            out=panel[:, jlo * 512:jlo * 512 + wlen],
                            in_=wt[:, :wlen],
                            func=ExpF,
                            scale=1.0 / math.sqrt(H),
                            # single-wave blocks: skip the ~285ns ACT
                            # accumulator read; DVE reduces the fp16 panel
                            accum_out=None if nwaves == 1 else rsp[:, wv_i:wv_i + 1],
                        )
                    den = small.tile([128, 1], f32, name="den")
                    if nwaves > 1:
                        nc.vector.tensor_reduce(den, rsp[:, :nwaves], axis=AxX, op=AluAdd)
                    else:
                        # single-wave: mask applied post-exp (0/1 triangle on
                        # the fp16 panel) so exp never waits the mask; the
                        # reduce then sums the masked zeros
                        dpan = (nch - 1) * 512 + (c0 - (nch - 1) * 512)
                        nc.vector.tensor_mul(
                            panel[:, dpan:dpan + 128],
                            panel[:, dpan:dpan + 128], mskm,
                        )
                        nc.vector.tensor_reduce(
                            den, panel[:, :c0 + 128], axis=AxX, op=AluAdd
                        )
                    rden = small.tile([128, 1], f32, name="rden")
                    nc.vector.reciprocal(rden, den)
                    vt = small.tile([128, 64], z_store, name="vt")
                    nc.vector.tensor_scalar_mul(vt, vsb[:, cb, hs], rden)
                    pending.append((vt, panel, nch, cb))
                    if len(pending) > Z_LAG:
                        emit_z(pending.pop(0))
                    # dependency-free weight loads keep the PE activity
                    # monitor from re-throttling the clock during waits
                    for _ in range(FILL):
                        nc.tensor.ldweights(zero_t[:, :128])
                for item in pending:
                    emit_z(item)
                if h == 0:
                    scr["zT"] = None
                    for j in range(NCH):
                        copy_zchunk(j)
                else:
                    for piece in out_pieces:
                        emit_out_piece(*piece, late=True)

    nc.compile()
    return nc


def get_program(S=S_FULL):
    if S not in _prog_cache:
        _prog_cache[S] = _build_program(S)
    return _prog_cache[S]


def make_mask_band():
    """Triangle mask for the last 128 cols of a diagonal chunk:
    col t (relative to the diagonal start) is valid iff t <= r."""
    r = np.arange(128)[:, None]
    t = np.arange(128)[None, :]
    return np.where(t <= r, 0.0, NEG_BIG).astype(np.float32)


def _pack_ec(a):
    """[E, W] -> [128, (E//128)*W] with ec chunks side by side:
    out[p, ec*W + c] = a[ec*128 + p, c]."""
    Edim, W = a.shape
    ec = Edim // 128
    return np.ascontiguousarray(
        a.reshape(ec, 128, W).transpose(1, 0, 2).reshape(128, ec * W)
    )


def make_core_inputs(x, W_K, W_Q, W_V, W_O, core):
    """Inputs for core = b*4 + g (batch b, head pair a0=2g, a1=2g+1)."""
    wire = np.float16 if PROJ_DT == "fp16" else np.float32
    b, g = divmod(core, 4)
    a0, a1 = 2 * g, 2 * g + 1
    xp = _pack_ec(np.ascontiguousarray(x[b].T))
    wk = _pack_ec(np.concatenate([W_K[a0].T, W_K[a1].T], axis=1))
    wq = _pack_ec(np.concatenate([W_Q[a0].T, W_Q[a1].T], axis=1))
    wv = _pack_ec(np.concatenate([W_V[a0].T, W_V[a1].T], axis=1))
    wo = np.ascontiguousarray(np.concatenate([W_O[a0].T, W_O[a1].T], axis=0))
    tri = make_mask_band()
    return {
        "xp": xp.astype(wire), "wk": wk.astype(wire), "wq": wq.astype(wire),
        "wv": wv.astype(wire), "wo": wo.astype(wire), "maskb": tri,
        "maskm": (tri == 0.0).astype(np.float16),
    }


def run_on_cores(inputs, trace=False):
    from concourse.bass_utils import run_bass_kernel_spmd

    nc = get_program()
    in_maps = [
        make_core_inputs(
            inputs["x"], inputs["W_K"], inputs["W_Q"], inputs["W_V"],
            inputs["W_O"], core,
        )
        for core in range(N_CORES)
    ]
    return run_bass_kernel_spmd(
        nc, in_maps, list(range(N_CORES)), trace=trace,
    )


def kernel(x, W_K, W_Q, W_V, W_O):
    x = np.asarray(x, dtype=np.float32)
    W_K = np.asarray(W_K, dtype=np.float32)
    W_Q = np.asarray(W_Q, dtype=np.float32)
    W_V = np.asarray(W_V, dtype=np.float32)
    W_O = np.asarray(W_O, dtype=np.float32)
    res = run_on_cores(
        {"x": x, "W_K": W_K, "W_Q": W_Q, "W_V": W_V, "W_O": W_O}
    )
    out = np.zeros((B, S_FULL, E), dtype=np.float32)
    for b in range(B):
        acc = np.zeros((E, S_FULL), dtype=np.float32)
        for g in range(4):
            acc += res.results[b * 4 + g]["outT"]
        out[b] = acc.T
    return out



# revision 10
# speedup vs baseline: 1.0156x; 1.0156x over previous
"""Trainium2 Bass kernel for the quirky multi-head attention problem.

Math (per batch b, head a), faithful to the reference:
    K = x[b] @ W_K[a].T          # [S, H]
    Q = x[b] @ W_Q[a].T
    V = x[b] @ W_V[a].T
    s[c, C] = (K @ Q.T)[c, C] / sqrt(H)        rows c = "key" index
    valid iff C <= c (tril); softmax over C per row c
    E = exp(s) * tril            # no max-subtraction: |s| <= ~7, fp32-safe
    denom[c] = sum_C E[c, C]
    z[C, h] = sum_c E[c, C] * (V/denom)[c, h]  # = E.T @ (V/denom)
    out[b] += z @ W_O[a].T
Sharding: 8 cores = 2 batches x 4 head-pairs. Each core handles one batch
and two heads; the attention matrix is device-local. Host sums the four
head-pair partial outputs per batch.

Device layouts (per core):
    xp   [128, EC, S] x[b] pre-packed on host: xp[p, ec, s] = x[s, ec*128+p]
    wk/wq/wv [128, EC*128] W[a0].T | W[a1].T, ec-chunks packed on partitions
    wo   [128, E]     W_O[a0].T on partitions 0:64, W_O[a1].T on 64:128
    maskb [128, 128]  additive causal mask band (0 or -1e9)
    outT [E, S]       partial output, transposed

On-chip flow per head: scores [c_blk=128 rows, 512-wide C chunks] are
matmul'd into rotating PSUM wave tiles (2 banks x 2 bufs), the diagonal
gets an additive -1e9 triangle mask, ScalarE applies exp (scale=1/sqrt(H))
writing the row panel to SBUF (fp16) with a fused per-row accumulation
(softmax denominator). z^T accumulates in PSUM across row blocks; chunk j
of C lives at partition half (j < NCH/2 ? 0 : 64) so z^T fits in 4 banks
and coexists with the wave tiles. Matmul dtypes: fp16 operands for the
attention matmuls (4e-4 rel err, 1 col/cycle like bf16), f32r (single-pass
fp32, now fp16 wire) for projections. z matmuls for block cb are emitted Z_LAG blocks
late so PE streams without stalling on the softmax chain; head 1 sweeps
rows in reverse so each z chunk j finalizes at block 4j and its copy +
merged-head output projection trickle through the sweep instead of
bunching in a cold epilogue.

DMA: weights are host-prepacked so each input is ONE large descriptor-
friendly transfer (the DGE generator serializes at ~650ns/copy); x streams
in a narrow 128-col first slice + 384-col second + seven 512-col eighths
so block 0's kt/qt chain starts ~6us in instead of ~16us.

Timing (TimelineSim cost model; HW ~= 1.195x sim per baseline calibration):
baseline 280237ns (HW 334882) -> this kernel 236653ns. x/weights/output
ship as fp16 on the wire (adds ~2e-4 rel err vs the 2e-3 gate, halves the
serialized DMA stream to ~25us total, and removes f32r's 4x penalty on the
narrow N=128 V-projection matmuls: PE busy drops 169->147us). ACT (exp)
busy ~170us is the single bottleneck at ~71% occupancy; remaining slack is
softmax-chain latency in head 0 (wave-buffer double-hold during projection
slices) and the reverse-sweep tail.
"""

import math

import numpy as np

B, S_FULL, E, A, H = 2, 4096, 512, 8, 64
N_CORES = 8
NEG_BIG = -1.0e9

import os as _os

ATTN_DT = _os.environ.get("ATTN_DT", "fp16")
PROJ_DT = _os.environ.get("PROJ_DT", "fp16")
OUT_DT = _os.environ.get("OUT_DT", "fp16")
Z_LAG = int(_os.environ.get("Z_LAG", "2"))
PANEL_BUFS = int(_os.environ.get("PANEL_BUFS", "15"))
EXTRA = int(_os.environ.get("EXTRA", "2"))       # backlog z pieces per block
VTP_BUFS = int(_os.environ.get("VTP_BUFS", "14"))
S1 = int(_os.environ.get("S1", "13"))            # deferred z starts (head 0)
S2 = int(_os.environ.get("S2", "17"))
S3 = int(_os.environ.get("S3", "21"))
ST_POOL = int(_os.environ.get("ST_POOL", "0"))   # out-proj st copies on Pool
KQ_POOL = int(_os.environ.get("KQ_POOL", "0"))   # kt/qt copies on Pool
KQ_ACT = int(_os.environ.get("KQ_ACT", "0"))     # kt/qt copies on ACT
FILL = int(_os.environ.get("FILL", "0"))
SMALL_BUFS = int(_os.environ.get("SMALL_BUFS", "8"))
OUTST_BUFS = int(_os.environ.get("OUTST_BUFS", "4"))         # dummy ldweights per block
OP_POPS = int(_os.environ.get("OP_POPS", "2"))   # out-proj pieces per block

_prog_cache = {}


def _build_program(S, attn_dt=None, proj_dt=None):
    import concourse.mybir as mybir
    import concourse.tile as tile
    from concourse import bacc

    attn_dt = attn_dt or ATTN_DT
    proj_dt = proj_dt or PROJ_DT
    f32 = mybir.dt.float32
    f32r = mybir.dt.float32r
    bf16 = mybir.dt.bfloat16
    fp16 = mybir.dt.float16
    att_store = {"bf16": bf16, "fp16": fp16, "f32r": f32r, "f32": f32}[attn_dt]
    z_store = {"bf16": bf16, "fp16": fp16, "f32r": fp16, "f32": f32}[attn_dt]
    proj_store = {"f32r": f32r, "f32": f32, "fp16": fp16}[proj_dt]
    out_store = {"f32": f32, "fp16": fp16}[OUT_DT]

    EC = E // 128            # e chunks (contraction for projections)
    NCB = S // 128           # row blocks
    NCH = S // 512           # C chunks per full row
    HALF = NCH // 2          # chunks per partition half of z^T
    assert NCH % 2 == 0

    nc = bacc.Bacc("TRN2", target_bir_lowering=False, debug=False)

    xp = nc.dram_tensor("xp", [128, EC, S], proj_store, kind="ExternalInput")
    wk = nc.dram_tensor("wk", [128, EC * 128], proj_store, kind="ExternalInput")
    wq = nc.dram_tensor("wq", [128, EC * 128], proj_store, kind="ExternalInput")
    wv = nc.dram_tensor("wv", [128, EC * 128], proj_store, kind="ExternalInput")
    wo = nc.dram_tensor("wo", [128, E], proj_store, kind="ExternalInput")
    maskb = nc.dram_tensor("maskb", [128, 128], f32, kind="ExternalInput")
    maskm = nc.dram_tensor("maskm", [128, 128], fp16, kind="ExternalInput")
    outT = nc.dram_tensor("outT", [E, S], out_store, kind="ExternalOutput")

    ExpF = mybir.ActivationFunctionType.Exp
    AxX = mybir.AxisListType.X
    AluAdd = mybir.AluOpType.add

    with tile.TileContext(nc) as tc:
        with (
            tc.tile_pool(name="singles", bufs=1) as singles,
            tc.tile_pool(name="panelp", bufs=PANEL_BUFS) as panelp,
            tc.tile_pool(name="small", bufs=SMALL_BUFS) as small,
            tc.tile_pool(name="vtp", bufs=VTP_BUFS) as vtp,
            tc.tile_pool(name="outst", bufs=OUTST_BUFS) as outst,
            tc.tile_pool(name="ps", bufs=2, space="PSUM") as ps,
            tc.tile_pool(name="zps", bufs=1, space="PSUM") as zps,
        ):
            # ---- load inputs: one big transfer per tensor, x in eighths,
            # ordered so the first projection inputs land first ----
            xt = singles.tile([128, EC, S], proj_store)
            wks = singles.tile([128, EC, 128], proj_store)
            wqs = singles.tile([128, EC, 128], proj_store)
            wvs = singles.tile([128, EC, 128], proj_store)
            wos = singles.tile([128, E], proj_store)
            msk = singles.tile([128, 128], f32)
            mskm = singles.tile([128, 128], fp16)

            nc.sync.dma_start(out=wks[:, :, :], in_=wk[:, :])
            nc.sync.dma_start(out=wqs[:, :, :], in_=wq[:, :])
            nc.sync.dma_start(out=wvs[:, :, :], in_=wv[:, :])
            SQ = S // 8
            # first slice narrow (128 cols) so block 0's kt/qt chain starts
            # ~3us earlier; masks right after it (needed by block 0's exp)
            xsl = [(0, 128), (128, 512)] + [(q * SQ, (q + 1) * SQ) for q in range(1, 8)]
            for i, (lo, hi) in enumerate(xsl):
                nc.sync.dma_start(out=xt[:, :, lo:hi], in_=xp[:, :, lo:hi])
                if i == 0:
                    nc.sync.dma_start(out=msk, in_=maskb[:, :])
                    nc.sync.dma_start(out=mskm, in_=maskm[:, :])
            nc.sync.dma_start(out=wos, in_=wo[:, :])
            # bf16 always: the K=1 zeroing matmuls are invalid ISA in f32r,
            # and mixing dtypes across an accumulation group is fine.
            zero_t = singles.tile([1, 576], bf16)
            nc.vector.memset(zero_t, 0.0)

            # ---- projections (emitted per x-quarter, interleaved into the
            # head-0 narrow blocks so PE stays dense while DMA streams in) ----
            kt = singles.tile([128, S], att_store)
            qt = singles.tile([128, S], att_store)
            vsb = singles.tile([128, NCB, 128], f32)
            CPQ = NCH // 4           # 512-chunks per x quarter

            scr = {"zT": None, "cb": 0, "rot": 0}

            # deferred z-start block per chunk (head 0): bank k is free as
            # projection scratch while cb < S_START[k] - 1, extending the
            # scratch windows so every kq chunk avoids borrowing a wave tile
            S_START = {0: 5, 1: S1, 2: S2, 3: S3}
            for _j in range(4, NCH):
                S_START[_j] = 4 * _j + 3

            def proj_wt(width):
                # head0 blocks < S_k-1 may scratch in zT bank k: that bank's
                # chunks start accumulating only at block S_k (lower, real
                # start=True) / its upper-half zero group is emitted at block
                # S_k too - both after any scratch use (WAR-ordered by tile)
                if scr["zT"] is not None:
                    avail = [k for k in (0, 1, 2, 3) if scr["cb"] < S_START[k] - 1]
                    if avail:
                        k = avail[scr["rot"] % len(avail)]
                        scr["rot"] += 1
                        return scr["zT"][:, k * 512:k * 512 + width], True
                return ps.tile([128, 1024], f32, tag="wave", name="wt")[:, :width], False

            def emit_kq_part(dst, w, lo, hi):
                wt, is_scr = proj_wt(hi - lo)
                for ec in range(EC):
                    nc.tensor.matmul(
                        wt, w[:, ec, :], xt[:, ec, lo:hi],
                        start=(ec == 0), stop=(ec == EC - 1),
                        skip_group_check=is_scr,
                    )
                if KQ_ACT:
                    nc.scalar.copy(dst[:, lo:hi], wt)
                else:
                    cp = nc.gpsimd if KQ_POOL else nc.vector
                    cp.tensor_copy(dst[:, lo:hi], wt)

            def emit_kq_chunk(dst, w, cc):
                emit_kq_part(dst, w, cc * 512, (cc + 1) * 512)

            def emit_v_block(cb):
                bsl = slice(cb * 128, (cb + 1) * 128)
                wt, is_scr = proj_wt(128)
                for ec in range(EC):
                    nc.tensor.matmul(
                        wt, xt[:, ec, bsl], wvs[:, ec, :],
                        start=(ec == 0), stop=(ec == EC - 1),
                        skip_group_check=is_scr,
                    )
                nc.vector.tensor_copy(vsb[:, cb, :], wt)

            def emit_proj_slice(cb):
                # spread projections across early row blocks, inside the
                # extended zT scratch windows, so kq chunks never borrow a
                # wave tile (which stalls the score->exp pipeline). Chunk c
                # needs x slice c which lands ~(2.1 + 0.9c) us in; block 2c
                # runs later than that throughout.
                #   block 0/1: split first chunk (narrow chain -> fast start)
                #   block 2c / 2c+1 (c=1..6): kt/qt chunk c
                #   block 15 / 16: kt/qt chunk 7 (after the last x slice)
                #   every block: its own vsb
                if cb == 0:
                    emit_kq_part(kt, wks, 0, 128)
                    emit_kq_part(qt, wqs, 0, 128)
                    emit_v_block(cb)
                    return
                # V first: its short copy drains while the kq chunk's longer
                # PSUM->SBUF copy overlaps the next block's scores
                emit_v_block(cb)
                if cb == 1:
                    emit_kq_part(kt, wks, 128, 512)
                    emit_kq_part(qt, wqs, 128, 512)
                elif 2 <= cb <= 13:
                    q, r = divmod(cb, 2)
                    if r == 0:
                        emit_kq_chunk(kt, wks, q)
                    else:
                        emit_kq_chunk(qt, wqs, q)
                elif cb == 15:
                    emit_kq_chunk(kt, wks, 7)
                elif cb == 16:
                    emit_kq_chunk(qt, wqs, 7)

            # zsb holds both heads' z in SBUF: head h on partitions
            # [64h, 64h+64), C chunk j at cols [512j, 512j+512). The output
            # projection then contracts both heads in ONE K=128 matmul.
            zsb = singles.tile([128, S], proj_store)

            # ---- attention per head ----
            for h in range(2):
                hs = slice(h * 64, (h + 1) * 64)
                # head 0 forward, head 1 reverse: in reverse order z chunk j
                # finalizes at block 4j, so its copy + output projection
                # trickle through the sweep
                order = list(range(NCB - 1, -1, -1)) if h == 1 else list(range(NCB))
                zT = zps.tile([128, HALF * 512], f32, name="zT")

                def emit_dummy(k):
                    # zero the upper partition half of z bank k (chunks
                    # j >= HALF accumulate with start=False onto these; a
                    # real start=True there would clear the lower half's
                    # bank bits)
                    nc.tensor.matmul(
                        zT[64:128, k * 512:(k + 1) * 512],
                        zero_t[:, :64], zero_t[:, 64:576],
                        start=True, stop=False, skip_group_check=True,
                    )

                if h == 1:
                    for k in range(HALF):
                        emit_dummy(k)
                else:
                    # head 0: bank k is scratch for projections until block
                    # S_k-2; its dummy is emitted at block S_k instead
                    scr["zT"] = zT

                first_cb = {}
                last_cb = {}
                for j in range(NCH):
                    part = [cb for cb in order if cb >= 4 * j]
                    first_cb[j] = part[0]
                    last_cb[j] = part[-1]

                def chunk_w(cb_i, j):
                    # valid width of chunk j in block cb_i's panel
                    return min(512, cb_i * 128 + 128 - j * 512)

                def copy_zchunk(j):
                    poff = 0 if j < HALF else 64
                    col = (j % HALF) * 512
                    nc.vector.tensor_copy(
                        zsb[h * 64:h * 64 + 64, j * 512:(j + 1) * 512],
                        zT[poff:poff + 64, col:col + 512],
                    )

                def emit_out_piece(ccn, ecn, late=False):
                    col = ccn * 512
                    esl = slice(ecn * 128, (ecn + 1) * 128)
                    if ccn < HALF:
                        # bank ccn of zT is fully copied out by now (chunk
                        # ccn just finished; chunk ccn+HALF finished 16
                        # blocks earlier in the reverse sweep) — reuse it so
                        # out-proj never steals a score wave buffer
                        wt = zT[:, (ccn % HALF) * 512:(ccn % HALF) * 512 + 512]
                    else:
                        wt = ps.tile([128, 1024], f32, tag="wave", name="wt")[:, :512]
                    nc.tensor.matmul(
                        wt, wos[:, esl], zsb[:, col:col + 512],
                        start=True, stop=True, skip_group_check=True,
                    )
                    st = outst.tile([128, 512], out_store, name="st")
                    if late:
                        # tail: ACT is idle once the exp stream dries up, and
                        # DVE's in-order queue would delay the next block's
                        # den chain behind this 658ns copy
                        nc.scalar.copy(st, wt)
                    else:
                        cp = nc.gpsimd if ST_POOL else nc.vector
                        cp.tensor_copy(st, wt)
                    nc.sync.dma_start(out=outT[esl, col:col + 512], in_=st)

                def emit_piece(j, vt_i, panel_i, cb_i, w):
                    # one z matmul: chunk j's contribution from block cb_i,
                    # exact width (the first emitted piece of a lower chunk
                    # is always full-width, so start=True covers the region)
                    poff = 0 if j < HALF else 64
                    col = (j % HALF) * 512
                    if h == 0:
                        start = (j < HALF) and j not in zfirst
                        zfirst.add(j)
                        stop = cb_i == NCB - 1
                    else:
                        start = (j < HALF) and cb_i == first_cb[j]
                        stop = cb_i == last_cb[j]
                    nc.tensor.matmul(
                        zT[poff:poff + 64, col:col + w],
                        vt_i,
                        panel_i[:, j * 512:j * 512 + w],
                        start=start, stop=stop,
                        skip_group_check=True,
                    )

                def emit_z(item):
                    # head 1 only: emit all chunk pieces of one block
                    vt_i, panel_i, nch_i, cb_i = item
                    for j in range(nch_i):
                        emit_piece(j, vt_i, panel_i, cb_i, chunk_w(cb_i, j))
                    # Head 1 runs in reverse: chunk j is final once cb=4j is
                    # done; trickle its copy + the merged output projection
                    # into the sweep, ONE ecn piece per block so out-proj
                    # never monopolizes the wave buffers.
                    if h == 1:
                        if cb_i % 4 == 0:
                            j = cb_i // 4
                            copy_zchunk(j)
                            out_pieces.extend((j, ecn) for ecn in range(EC))
                        if out_pieces:
                            emit_out_piece(*out_pieces.pop(0), late=cb_i <= 8)

                def queue_block(item):
                    # head 0 only: split a block into per-chunk pieces
                    vt_i, panel_i, nch_i, cb_i = item
                    for j in range(nch_i):
                        zq[j].append((j, vt_i, panel_i, cb_i, chunk_w(cb_i, j)))

                def drain(cur, budget):
                    # head 0 only: start chunks whose block has come, emit one
                    # piece per started chunk plus `budget` backlog pieces
                    for j in range(NCH):
                        if j not in zstarted and cur >= S_START[j]:
                            zstarted.add(j)
                            if j < HALF:
                                emit_dummy(j)
                            # widest first so the start=True piece covers the
                            # whole 512-col region
                            zq[j].sort(key=lambda p: -p[4])
                    for j in range(NCH):
                        if j in zstarted and zq[j]:
                            emit_piece(*zq[j].pop(0))
                    while budget > 0:
                        cand = max(
                            (jj for jj in zstarted if zq[jj]),
                            key=lambda jj: len(zq[jj]), default=None,
                        )
                        if cand is None:
                            break
                        emit_piece(*zq[cand].pop(0))
                        budget -= 1

                pending = []
                out_pieces = []
                zq = {j: [] for j in range(NCH)}
                zstarted = set()
                zfirst = set()
                for oi, cb in enumerate(order):
                    if h == 0:
                        scr["cb"] = cb
                        emit_proj_slice(cb)
                    c0 = cb * 128
                    nch = (c0 + 128 + 511) // 512
                    nwaves = (nch + 1) // 2
                    lastw = c0 + 128 - (nch - 1) * 512   # width of diag chunk
                    panel = panelp.tile([128, S], z_store, name="panel")
                    rsp = small.tile([128, 4], f32, name="rsp")
                    for wv_i in range(nwaves):
                        jlo = 2 * wv_i
                        jhi = min(jlo + 2, nch)
                        wt = ps.tile([128, 1024], f32, tag="wave", name="wt")
                        for j in range(jlo, jhi):
                            w_n = lastw if j == nch - 1 else 512
                            nc.tensor.matmul(
                                wt[:, (j - jlo) * 512:(j - jlo) * 512 + w_n],
                                kt[hs, c0:c0 + 128],
                                qt[hs, j * 512:j * 512 + w_n],
                                start=True, stop=True,
                            )
                        if jhi == nch and nwaves > 1:
                            # mask only the last 128 cols (the true triangle);
                            # earlier diag-chunk cols are fully valid
                            o = c0 - (nch - 1) * 512
                            dlo = (nch - 1 - jlo) * 512 + o
                            nc.vector.tensor_add(
                                wt[:, dlo:dlo + 128], wt[:, dlo:dlo + 128],
                                msk,
                            )
                        wlen = (jhi - jlo - 1) * 512 + (lastw if jhi == nch else 512)
                        nc.scalar.activation(
                            out=panel[:, jlo * 512:jlo * 512 + wlen],
                            in_=wt[:, :wlen],
                            func=ExpF,
                            scale=1.0 / math.sqrt(H),
                            # single-wave blocks: skip the ~285ns ACT
                            # accumulator read; DVE reduces the fp16 panel
                            accum_out=None if nwaves == 1 else rsp[:, wv_i:wv_i + 1],
                        )
                    den = small.tile([128, 1], f32, name="den")
                    if nwaves > 1:
                        nc.vector.tensor_reduce(den, rsp[:, :nwaves], axis=AxX, op=AluAdd)
                    else:
                        # single-wave: mask applied post-exp (0/1 triangle on
                        # the fp16 panel) so exp never waits the mask; the
                        # reduce then sums the masked zeros
                        dpan = (nch - 1) * 512 + (c0 - (nch - 1) * 512)
                        nc.vector.tensor_mul(
                            panel[:, dpan:dpan + 128],
                            panel[:, dpan:dpan + 128], mskm,
                        )
                        nc.vector.tensor_reduce(
                            den, panel[:, :c0 + 128], axis=AxX, op=AluAdd
                        )
                    rden = small.tile([128, 1], f32, name="rden")
                    nc.vector.reciprocal(rden, den)
                    vt = (vtp if h == 0 else small).tile(
                        [128, 64], z_store, name="vt"
                    )
                    nc.vector.tensor_scalar_mul(vt, vsb[:, cb, hs], rden)
                    pending.append((vt, panel, nch, cb))
                    if len(pending) > Z_LAG:
                        item = pending.pop(0)
                        if h == 0:
                            queue_block(item)
                        else:
                            emit_z(item)
                    if h == 0:
                        drain(cb, EXTRA)
                    # dependency-free weight loads keep the PE activity
                    # monitor from re-throttling the clock during waits
                    for _ in range(FILL):
                        nc.tensor.ldweights(zero_t[:, :128])
                for item in pending:
                    if h == 0:
                        queue_block(item)
                    else:
                        emit_z(item)
                if h == 0:
                    drain(NCB, 10 ** 9)
                    scr["zT"] = None
                    for j in range(NCH):
                        copy_zchunk(j)
                else:
                    for piece in out_pieces:
                        emit_out_piece(*piece, late=True)

    nc.compile()
    return nc


def get_program(S=S_FULL):
    if S not in _prog_cache:
        _prog_cache[S] = _build_program(S)
    return _prog_cache[S]


def make_mask_band():
    """Triangle mask for the last 128 cols of a diagonal chunk:
    col t (relative to the diagonal start) is valid iff t <= r."""
    r = np.arange(128)[:, None]
    t = np.arange(128)[None, :]
    return np.where(t <= r, 0.0, NEG_BIG).astype(np.float32)


def _pack_ec(a):
    """[E, W] -> [128, (E//128)*W] with ec chunks side by side:
    out[p, ec*W + c] = a[ec*128 + p, c]."""
    Edim, W = a.shape
    ec = Edim // 128
    return np.ascontiguousarray(
        a.reshape(ec, 128, W).transpose(1, 0, 2).reshape(128, ec * W)
    )


def make_core_inputs(x, W_K, W_Q, W_V, W_O, core):
    """Inputs for core = b*4 + g (batch b, head pair a0=2g, a1=2g+1)."""
    wire = np.float16 if PROJ_DT == "fp16" else np.float32
    b, g = divmod(core, 4)
    a0, a1 = 2 * g, 2 * g + 1
    xp = _pack_ec(np.ascontiguousarray(x[b].T))
    wk = _pack_ec(np.concatenate([W_K[a0].T, W_K[a1].T], axis=1))
    wq = _pack_ec(np.concatenate([W_Q[a0].T, W_Q[a1].T], axis=1))
    wv = _pack_ec(np.concatenate([W_V[a0].T, W_V[a1].T], axis=1))
    wo = np.ascontiguousarray(np.concatenate([W_O[a0].T, W_O[a1].T], axis=0))
    tri = make_mask_band()
    return {
        "xp": xp.astype(wire), "wk": wk.astype(wire), "wq": wq.astype(wire),
        "wv": wv.astype(wire), "wo": wo.astype(wire), "maskb": tri,
        "maskm": (tri == 0.0).astype(np.float16),
    }


def run_on_cores(inputs, trace=False):
    from concourse.bass_utils import run_bass_kernel_spmd

    nc = get_program()
    in_maps = [
        make_core_inputs(
            inputs["x"], inputs["W_K"], inputs["W_Q"], inputs["W_V"],
            inputs["W_O"], core,
        )
        for core in range(N_CORES)
    ]
    return run_bass_kernel_spmd(
        nc, in_maps, list(range(N_CORES)), trace=trace,
    )


def kernel(x, W_K, W_Q, W_V, W_O):
    x = np.asarray(x, dtype=np.float32)
    W_K = np.asarray(W_K, dtype=np.float32)
    W_Q = np.asarray(W_Q, dtype=np.float32)
    W_V = np.asarray(W_V, dtype=np.float32)
    W_O = np.asarray(W_O, dtype=np.float32)
    res = run_on_cores(
        {"x": x, "W_K": W_K, "W_Q": W_Q, "W_V": W_V, "W_O": W_O}
    )
    out = np.zeros((B, S_FULL, E), dtype=np.float32)
    for b in range(B):
        acc = np.zeros((E, S_FULL), dtype=np.float32)
        for g in range(4):
            acc += res.results[b * 4 + g]["outT"]
        out[b] = acc.T
    return out

